# revision 22
# baseline (speedup 1.0000x reference)
"""CapsuleLayer (dynamic routing) Trainium2 Bass kernel.

x (128, 1152, 8) f32, W (1152, 32, 8, 16) f32 ->
  u_hat = einsum('bid,ijdk->bijk'); 3 routing iterations -> v (128, 32, 16).

Batch-sharded over 8 cores (16 b per core), W replicated, routing local.

Per core:
  phase 1: u_hat bf16 in SBUF, layout [p=(i%8)*16+b, f=(i//8, j, k)],
    via 144 matmuls with block-diagonal-x stationaries
    (K=(il,d)=64, M=(il,b)=128, N=(j,k)=512); iteration-0 weighted sum
    s0 = (1/32) sum_i u_hat via 144 dense accumulating matmuls (K=64,M=16).
  iterations 1,2:
    agreement b_ij = sum_k u*Vsum : DVE mul (bf16 2x) + TT-tree reduce.
    softmax over j: ACT exp + DVE reduce/reciprocal.
    s = sum_i c*u : DVE mul c (pair-packed bf16) then PE accumulation with
    a ones-delta stationary (contracts the partition dim (il,b) -> b).
    squash on [16, 512] tiles.
"""

import numpy as np

B = 128
BL = 16  # batch per core
I = 1152
J = 32
D = 8
K = 16
JK = J * K  # 512
NCORES = 8
NG = I // 8      # 144 groups of 8 i's
GSUB = 24        # routing sub-blocks
GPS = NG // GSUB  # 9 groups per sub-block

_cached = {}
_last_in_maps = None


def _build_bass():
    import concourse.bass as bass
    import concourse.bacc as bacc_mod
    import concourse.tile as tile
    from concourse import mybir

    fp32 = mybir.dt.float32
    bf16 = mybir.dt.bfloat16
    AF = mybir.ActivationFunctionType
    ALU = mybir.AluOpType
    AX = mybir.AxisListType

    nc = bacc_mod.Bacc()

    # per-group packed input: [wt (512) | xbd (128) | xs (16)] per partition row
    q_d = nc.declare_dram_parameter("q", [NG // 2, 128, JK + 128 + BL], bf16, isOutput=False)
    ones_rep_d = nc.declare_dram_parameter("ones_rep", [BL, 128], fp32, isOutput=False)
    ones_sum_d = nc.declare_dram_parameter("ones_sum", [128, BL], bf16, isOutput=False)
    out_d = nc.declare_dram_parameter("out", [BL, J, K], fp32, isOutput=True)

    with tile.TileContext(nc) as tc:
        with (
            tc.tile_pool(name="big", bufs=1) as big,
            tc.tile_pool(name="consts", bufs=1) as consts,
            tc.tile_pool(name="wt", bufs=4) as wtp,
            tc.tile_pool(name="scr", bufs=2) as scr,
            tc.tile_pool(name="small", bufs=2) as small,
            tc.tile_pool(name="p2", bufs=3) as p2p,
            tc.tile_pool(name="pmul", bufs=3) as pmulp,
            tc.tile_pool(name="psum_u", bufs=4, space="PSUM") as psum_u_p,
            tc.tile_pool(name="psum_acc", bufs=1, space="PSUM") as psum_acc_p,
            tc.tile_pool(name="psum_misc", bufs=2, space="PSUM") as psum_misc_p,
        ):
            # ---------------- constants / staging ----------------
            ones_rep = consts.tile([BL, 128], fp32)   # delta[b, (il,b')]
            nc.gpsimd.dma_start(out=ones_rep, in_=ones_rep_d[:, :])
            ones_sum = consts.tile([128, BL], bf16)   # delta[(il,b), b']
            nc.gpsimd.dma_start(out=ones_sum, in_=ones_sum_d[:, :])

            U1 = big.tile([128, NG, J, K], bf16)

            psum_s0 = psum_acc_p.tile([BL, JK], fp32)

            # ---------------- phase 1 ----------------
            QW = JK + 128 + BL  # 656
            QB = 2  # pairs per DMA batch (4 groups)
            NP = NG // 2  # 72 pairs
            for pb in range(NP // QB):
                qt = wtp.tile([128, QB, QW], bf16)
                nc.sync.dma_start(
                    out=qt, in_=q_d[pb * QB : (pb + 1) * QB].transpose([1, 0, 2])
                )
                for h in range(QB):
                    pp = pb * QB + h
                    # two concurrent u-matmuls on row-group halves
                    for half in range(2):
                        g = 2 * pp + half
                        sl = slice(64 * half, 64 * half + 64)
                        wt = qt[sl, h, 0:JK]
                        xbd = qt[sl, h, JK : JK + 128]
                        pu = psum_u_p.tile([128, JK], fp32)
                        nc.tensor.matmul(
                            pu, xbd, wt, start=True, stop=True,
                            tile_position=(64 * half, 0),
                        )
                        dst = U1[:, g].rearrange("p j k -> p (j k)")
                        if g % 3 == 0:
                            nc.vector.tensor_copy(out=dst, in_=pu)
                        else:
                            nc.scalar.copy(out=dst, in_=pu)
                    # s0 partial over both groups (K=128)
                    xs = qt[:, h, JK + 128 : JK + 128 + BL]
                    wtf = qt[:, h, 0:JK]
                    nc.tensor.matmul(
                        psum_s0, xs, wtf, start=(pp == 0), stop=(pp == NP - 1),
                        skip_group_check=True,
                    )

            eps_tile = consts.tile([BL, 1], fp32)
            nc.vector.memset(eps_tile, 1e-9)

            # ---------------- squash helper ----------------
            def squash(psum_s, scale, vout_f32):
                s_sb = small.tile([BL, J, K], fp32)
                nc.scalar.activation(
                    out=s_sb.rearrange("b j k -> b (j k)"),
                    in_=psum_s,
                    func=AF.Copy,
                    scale=float(scale),
                )
                s2 = small.tile([BL, J, K], fp32)
                nc.vector.tensor_mul(s2, s_sb, s_sb)
                sq = small.tile([BL, J], fp32)
                nc.vector.tensor_reduce(out=sq, in_=s2, axis=AX.X, op=ALU.add)
                rt = small.tile([BL, J], fp32)
                nc.scalar.activation(out=rt, in_=sq, func=AF.Sqrt, bias=eps_tile[:, :])
                den = small.tile([BL, J], fp32)
                nc.vector.tensor_mul(den, sq, rt)
                nc.vector.tensor_add(den, den, rt)
                rec = small.tile([BL, J], fp32)
                nc.vector.reciprocal(out=rec, in_=den)
                fac = small.tile([BL, J], fp32)
                nc.vector.tensor_mul(fac, sq, rec)
                fac_b = fac[:, :].unsqueeze(2).to_broadcast([BL, J, K])
                nc.vector.tensor_tensor(out=vout_f32, in0=s_sb, in1=fac_b, op=ALU.mult)

            v_f32 = consts.tile([BL, J, K], fp32)
            Vsum = consts.tile([BL, J, K], fp32)
            squash(psum_s0, 1.0 / J, v_f32)
            nc.vector.tensor_copy(out=Vsum, in_=v_f32)

            vrep = consts.tile([128, J, K], bf16)  # Vsum replicated to (il,b)

            def build_vrep():
                pv = psum_misc_p.tile([128, JK], fp32)
                nc.tensor.matmul(
                    pv, ones_rep, Vsum.rearrange("b j k -> b (j k)"),
                    start=True, stop=True,
                )
                nc.scalar.copy(out=vrep.rearrange("p j k -> p (j k)"), in_=pv)

            build_vrep()

            # ---------------- routing iterations ----------------
            for it in (1, 2):
                psum_s = psum_acc_p.tile([BL, JK], fp32)
                nmm = 0
                for sub in range(GSUB):
                    g0 = sub * GPS
                    # agreement: prod = U1_sub * vrep  (broadcast over g)
                    prod = scr.tile([128, GPS, J, K], bf16)
                    vrep_b = vrep[:, :, :].unsqueeze(1).to_broadcast(
                        [128, GPS, J, K]
                    )
                    nc.vector.tensor_tensor(
                        out=prod, in0=U1[:, g0 : g0 + GPS], in1=vrep_b, op=ALU.mult
                    )
                    nc.vector.tensor_tensor(
                        out=prod[:, :, :, 0:8], in0=prod[:, :, :, 0:8],
                        in1=prod[:, :, :, 8:16], op=ALU.add,
                    )
                    nc.vector.tensor_tensor(
                        out=prod[:, :, :, 0:4], in0=prod[:, :, :, 0:4],
                        in1=prod[:, :, :, 4:8], op=ALU.add,
                    )
                    bij = p2p.tile([128, GPS, J], fp32)
                    nc.vector.tensor_reduce(
                        out=bij, in_=prod[:, :, :, 0:4], axis=AX.X, op=ALU.add
                    )
                    # softmax over j (exp in place, ACT accumulates Z)
                    e = bij
                    z = p2p.tile([128, GPS], fp32)
                    for gg in range(GPS):
                        nc.scalar.activation(
                            out=e[:, gg], in_=bij[:, gg], func=AF.Exp,
                            accum_out=z[:, gg : gg + 1],
                        )
                    rz = z
                    nc.vector.reciprocal(out=rz, in_=z)
                    # c stored as adjacent pairs: c2[p, g, j, 2]
                    c2 = p2p.tile([128, GPS, J, 2], bf16)
                    e_b = e[:, :, :].unsqueeze(3).to_broadcast([128, GPS, J, 2])
                    rz_b2 = (
                        rz[:, :].unsqueeze(2).unsqueeze(3)
                        .to_broadcast([128, GPS, J, 2])
                    )
                    nc.vector.tensor_tensor(
                        out=c2, in0=e_b, in1=rz_b2, op=ALU.mult
                    )
                    # weighted sum: p2 = c * u (one batched mul); PE-sum
                    p2 = pmulp.tile([128, GPS, J, K], bf16)
                    c_all = c2[:, :]  # [128, GPS, J, 2]
                    c_b = bass.AP(
                        tensor=c_all.tensor,
                        offset=c_all.offset,
                        ap=[c_all.ap[0], c_all.ap[1], c_all.ap[2],
                            [0, K // 2], [1, 2]],
                    )
                    nc.vector.tensor_tensor(
                        out=p2.rearrange(
                            "p g j (kk two) -> p g j kk two", two=2
                        ),
                        in0=U1[:, g0 : g0 + GPS].rearrange(
                            "p g j (kk two) -> p g j kk two", two=2
                        ),
                        in1=c_b,
                        op=ALU.mult,
                    )
                    for gg in range(GPS):
                        nc.tensor.matmul(
                            psum_s,
                            ones_sum,
                            p2[:, gg].rearrange("p j k -> p (j k)"),
                            start=(nmm == 0),
                            stop=(nmm == NG - 1),
                            skip_group_check=True,
                        )
                        nmm += 1
                squash(psum_s, 1.0, v_f32)
                if it < 2:
                    nc.vector.tensor_add(Vsum, Vsum, v_f32)
                    build_vrep()

            nc.sync.dma_start(out=out_d[:, :, :], in_=v_f32)

    nc.finalize()
    return nc


def kernel(x: np.ndarray, W: np.ndarray) -> np.ndarray:
    from concourse.bass_utils import run_bass_kernel_spmd

    if "nc" not in _cached:
        _cached["nc"] = _build_bass()
    nc = _cached["nc"]

    x = np.ascontiguousarray(x, dtype=np.float32)
    W = np.ascontiguousarray(W, dtype=np.float32)
    # W as [g, (il d), (j k)]
    w_t = W.transpose(0, 2, 1, 3).reshape(NG, 64, JK)

    ones_rep = np.zeros((BL, 128), dtype=np.float32)
    for b in range(BL):
        for il in range(8):
            ones_rep[b, il * BL + b] = 1.0
    import ml_dtypes
    ones_sum = np.ascontiguousarray(ones_rep.T).astype(ml_dtypes.bfloat16)

    in_maps = []
    for c in range(NCORES):
        xl = x[c * BL : (c + 1) * BL]  # [16, 1152, 8]
        blocks = xl.reshape(BL, NG, 8, D).transpose(1, 2, 3, 0)  # [g, il, d, b]
        xs_all = blocks.reshape(NG, 64, BL)
        xbd_all = np.zeros((NG, 8, D, 8, BL), dtype=np.float32)
        for il in range(8):
            xbd_all[:, il, :, il, :] = blocks[:, il]
        xbd_all = xbd_all.reshape(NG, 64, 128)
        import ml_dtypes
        q = np.concatenate([w_t, xbd_all, xs_all], axis=2)
        q = np.ascontiguousarray(
            q.reshape(NG // 2, 2 * 64, JK + 128 + BL)
        ).astype(ml_dtypes.bfloat16)
        in_maps.append(
            {
                "q": q,
                "ones_rep": ones_rep,
                "ones_sum": ones_sum,
            }
        )
    global _last_in_maps
    _last_in_maps = in_maps
    res = run_bass_kernel_spmd(nc, in_maps, core_ids=list(range(NCORES)))
    outs = [res.results[c]["out"] for c in range(NCORES)]
    return np.concatenate(outs, axis=0).astype(np.float32)


if __name__ == "__main__":
    rng = np.random.default_rng(0)
    x = rng.standard_normal((B, I, D), dtype=np.float32)
    W = (rng.standard_normal((I, J, D, K)) * np.sqrt(2.0 / 24)).astype(np.float32)
    v = kernel(x, W)
    print(v.shape, v.dtype, float(np.abs(v).mean()))
